# revision 9
# baseline (speedup 1.0000x reference)
"""DetectionLoss Trainium2 kernel: 8-core data-parallel (4 images/core).

Device computes, per image, partial sums over anchors ([128,6] per image):
  [npos, nneg, sum(ce_bg), sum(ce_tgt*posf), sum(ce_bg*negf), -sum(sl*posf)]
Host finishes the scalar combine exactly as the reference does.

Matched-GT gather runs on the PE: transpose the argmax tie-mask, then
matmul against per-GT [gcx, gcy, log(gw), log(gh), label] columns.
"""
import os
import sys
import numpy as np

sys.path.insert(0, "/opt/trn_rl_repo")

import concourse.bass as bass
import concourse.bacc as bacc
import concourse.mybir as mybir
from concourse import tile
from concourse.bass_utils import run_bass_kernel_spmd

F32 = mybir.dt.float32
ALU = mybir.AluOpType
ACT = mybir.ActivationFunctionType
AX = mybir.AxisListType

P = 128          # partitions
G = 200          # free columns per anchor plane (N = P*G = 25600)
N = P * G
M = 50           # max GT boxes
C = 8            # classes
BPC = 4          # images per core
NCORES = 8
GPC = 25         # groups per pair-stage chunk
NCHUNK = G // GPC
NQ = 5           # gathered per-GT quantities

# anchor plane indices in the "anc" DRAM tensor
A_CXM, A_CYM, A_WH, A_HH, A_W, A_H, A_I2W, A_I2H, A_LW, A_LH, A_CX, A_CY = range(12)
NANC = 12
# gt plane indices (each M wide) in "gt"
G_X1, G_Y1, G_X2, G_Y2, G_A2 = range(5)
NGT = 5
NOUT = 6


def _rep_last(ap, n):
    """[..., d] -> [..., d, n] with step-0 broadcast."""
    return bass.AP(ap.tensor, ap.offset, list(ap.ap) + [[0, n]])


def _rep_mid(ap, n):
    """[p, d] -> [p, n, d] with step-0 broadcast."""
    a = list(ap.ap)
    return bass.AP(ap.tensor, ap.offset, [a[0], [0, n]] + a[1:])


def _v3(ap2d):
    """[128, 200] plane -> [128, 8, 25]."""
    return ap2d.rearrange("p (u v) -> p u v", v=GPC)


def build_program():
    nc = bacc.Bacc(None, target_bir_lowering=False)
    cls_d = nc.dram_tensor("cls", [BPC, P, C * G], F32, kind="ExternalInput")
    reg_d = nc.dram_tensor("reg", [BPC, P, 4 * G], F32, kind="ExternalInput")
    anc_d = nc.dram_tensor("anc", [P, NANC * G], F32, kind="ExternalInput")
    gt_d = nc.dram_tensor("gt", [BPC, P, NGT * M], F32, kind="ExternalInput")
    gt5_d = nc.dram_tensor("gt5", [BPC, 64, 8], F32, kind="ExternalInput")
    iden_d = nc.dram_tensor("iden", [P, P], F32, kind="ExternalInput")
    res_d = nc.dram_tensor("res", [BPC, P, NOUT], F32, kind="ExternalOutput")

    with tile.TileContext(nc) as tc:
        with (
            tc.tile_pool(name="const", bufs=1) as cpool,
            tc.tile_pool(name="img", bufs=2) as ipool,
            tc.tile_pool(name="work", bufs=2) as wpool,
            tc.tile_pool(name="psum", bufs=2, space="PSUM") as ppool,
            tc.tile_pool(name="out", bufs=2) as opool,
        ):
            anc = cpool.tile([P, NANC * G], F32)
            nc.sync.dma_start(anc[:], anc_d[:])
            iden = cpool.tile([P, P], F32)
            nc.sync.dma_start(iden[:], iden_d[:])

            def ancp(k):
                return anc[:, k * G:(k + 1) * G]

            for b in [bb for _ in range(int(os.environ.get('DETLOSS_REPS', '1'))) for bb in range(BPC)]:
                ct = ipool.tile([P, C * G], F32, tag="ct", name="ct")
                nc.sync.dma_start(ct[:], cls_d[b])
                rt = ipool.tile([P, 4 * G], F32, tag="rt", name="rt")
                nc.sync.dma_start(rt[:], reg_d[b])
                gtt = ipool.tile([P, NGT * M], F32, tag="gtt", name="gtt")
                nc.sync.dma_start(gtt[:], gt_d[b])
                gtq = ipool.tile([P, 8], F32, tag="gtq", name="gtq")
                nc.sync.dma_start(gtq[0:64, :], gt5_d[b])
                # gathered per-GT quantities, chunk-major [8 x [128, 25*5]]
                pg5 = ipool.tile([P, NCHUNK * GPC * NQ], F32, tag="pg5", name="pg5")

                def g5(q):
                    """[128, 8, 25] strided view of gathered quantity q."""
                    a = pg5[:, :]
                    return bass.AP(a.tensor, a.offset + q,
                                   [a.ap[0], [GPC * NQ, NCHUNK], [NQ, GPC]])

                def clsp(k):
                    return ct[:, k * G:(k + 1) * G]

                def regp(k):
                    return rt[:, k * G:(k + 1) * G]

                def gtp(k):
                    return gtt[:, k * M:(k + 1) * M]

                def it(tag):
                    return ipool.tile([P, G], F32, tag=tag, name=tag)

                # ---- decode boxes ----
                cx = it("cx"); cy = it("cy"); w = it("w"); h = it("h")
                ew = it("ew"); hw = it("hw")
                x1 = it("x1"); x2 = it("x2"); y1 = it("y1"); y2 = it("y2")
                a1 = it("a1")
                nc.vector.tensor_tensor(cx[:], regp(0), ancp(A_WH), ALU.mult)
                nc.vector.tensor_tensor(cx[:], cx[:], ancp(A_CXM), ALU.add)
                nc.vector.tensor_tensor(cy[:], regp(1), ancp(A_HH), ALU.mult)
                nc.vector.tensor_tensor(cy[:], cy[:], ancp(A_CYM), ALU.add)
                nc.scalar.activation(ew[:], regp(2), ACT.Exp)
                nc.vector.tensor_tensor(w[:], ew[:], ancp(A_W), ALU.mult)
                nc.scalar.activation(ew[:], regp(3), ACT.Exp)
                nc.vector.tensor_tensor(h[:], ew[:], ancp(A_H), ALU.mult)
                nc.scalar.activation(hw[:], w[:], ACT.Copy, scale=0.5)
                nc.vector.tensor_sub(x1[:], cx[:], hw[:])
                nc.vector.tensor_add(x2[:], cx[:], hw[:])
                nc.scalar.activation(hw[:], h[:], ACT.Copy, scale=0.5)
                nc.vector.tensor_sub(y1[:], cy[:], hw[:])
                nc.vector.tensor_add(y2[:], cy[:], hw[:])
                nc.vector.tensor_mul(a1[:], w[:], h[:])

                # ---- pair stage: per-anchor max IoU + matched-GT gather ----
                mx = it("mx")
                gx1b = _rep_mid(gtp(G_X1), GPC)
                gy1b = _rep_mid(gtp(G_Y1), GPC)
                gx2b = _rep_mid(gtp(G_X2), GPC)
                gy2b = _rep_mid(gtp(G_Y2), GPC)
                a2b = _rep_mid(gtp(G_A2), GPC)

                for k in range(NCHUNK):
                    g0 = k * GPC
                    sl = slice(g0, g0 + GPC)

                    def wt(tag):
                        t = wpool.tile([P, GPC * M], F32, tag=tag, name=tag)
                        return t, t[:].rearrange("p (g m) -> p g m", m=M)

                    ta, tav = wt("ta"); tb, tbv = wt("tb"); tcn, tcv = wt("tc")
                    td, tdv = wt("td"); te, tev = wt("te"); tf, tfv = wt("tf")

                    nc.vector.tensor_tensor(tav, _rep_last(x1[:, sl], M), gx1b, ALU.max)
                    nc.vector.tensor_tensor(tbv, _rep_last(x2[:, sl], M), gx2b, ALU.min)
                    nc.vector.tensor_tensor(tcv, tbv, tav, ALU.subtract)
                    nc.vector.tensor_tensor(tav, _rep_last(y1[:, sl], M), gy1b, ALU.max)
                    nc.vector.tensor_tensor(tbv, _rep_last(y2[:, sl], M), gy2b, ALU.min)
                    nc.vector.tensor_tensor(tdv, tbv, tav, ALU.subtract)
                    nc.scalar.activation(ta[:], tcn[:], ACT.Relu)
                    nc.scalar.activation(tb[:], td[:], ACT.Relu)
                    nc.vector.tensor_mul(tcn[:], ta[:], tb[:])          # inter
                    nc.vector.tensor_tensor(tdv, _rep_last(a1[:, sl], M), a2b, ALU.add)
                    nc.vector.tensor_sub(te[:], td[:], tcn[:])          # union
                    nc.vector.reciprocal(tb[:], te[:])
                    nc.vector.tensor_mul(tf[:], tcn[:], tb[:])          # iou
                    nc.vector.reduce_max(mx[:, sl], tfv, axis=AX.X)
                    nc.vector.tensor_tensor(tav, tfv, _rep_last(mx[:, sl], M),
                                            ALU.is_equal)               # tie-mask
                    # PE gather: out[anchor, q] = sum_gt mask * gtq
                    pout = ppool.tile([P, GPC * NQ], F32, tag="pout", name="pout")
                    for g in range(GPC):
                        w0 = g * M
                        psT = ppool.tile([M, P], F32, tag="psT", name="psT")
                        nc.tensor.transpose(psT[:], ta[:, w0:w0 + M], iden[:])
                        ohT = wpool.tile([M, P], F32, tag="ohT", name="ohT")
                        nc.scalar.activation(ohT[:], psT[:], ACT.Copy)
                        nc.tensor.matmul(pout[:, g * NQ:(g + 1) * NQ],
                                         ohT[:], gtq[0:M, 0:NQ])
                    nc.scalar.activation(pg5[:, k * GPC * NQ:(k + 1) * GPC * NQ],
                                         pout[:], ACT.Copy)

                # ---- classification ----
                acc = it("acc"); tmp = it("tmp"); tmp2 = it("tmp2")
                lse = it("lse"); bgt = it("bgt"); xt = it("xt"); tgtt = it("tgtt")
                posf = it("posf"); negf = it("negf"); scr = it("scr")
                for c in range(C):
                    if c == 0:
                        nc.scalar.activation(acc[:], clsp(0), ACT.Exp)
                    else:
                        nc.scalar.activation(tmp2[:], clsp(c), ACT.Exp)
                        nc.vector.tensor_add(acc[:], acc[:], tmp2[:])
                nc.scalar.activation(lse[:], acc[:], ACT.Ln)
                nc.vector.tensor_sub(bgt[:], lse[:], clsp(0))           # ce_bg
                labv = g5(4)
                for c in range(C):
                    if c == 0:
                        nc.vector.scalar_tensor_tensor(
                            _v3(xt[:]), labv, 0.0, _v3(clsp(0)), ALU.is_equal, ALU.mult)
                    else:
                        nc.vector.scalar_tensor_tensor(
                            _v3(tmp[:]), labv, float(c), _v3(clsp(c)),
                            ALU.is_equal, ALU.mult)
                        nc.vector.tensor_add(xt[:], xt[:], tmp[:])
                nc.vector.tensor_sub(tgtt[:], lse[:], xt[:])            # ce_tgt
                nc.vector.tensor_scalar(posf[:], mx[:], 0.25, None, ALU.is_ge)
                nc.vector.tensor_scalar(negf[:], mx[:], 0.1, None, ALU.is_lt)

                ot = opool.tile([P, NOUT], F32, tag="ot", name="ot")
                nc.scalar.activation(scr[:], posf[:], ACT.Copy, accum_out=ot[:, 0:1])
                nc.scalar.activation(scr[:], negf[:], ACT.Copy, accum_out=ot[:, 1:2])
                nc.scalar.activation(scr[:], bgt[:], ACT.Copy, accum_out=ot[:, 2:3])
                nc.vector.tensor_mul(scr[:], tgtt[:], posf[:])
                nc.vector.reduce_sum(ot[:, 3:4],
                                     scr[:].rearrange("p (g m) -> p g m", g=1), axis=AX.X)
                nc.vector.tensor_mul(scr[:], bgt[:], negf[:])
                nc.vector.reduce_sum(ot[:, 4:5],
                                     scr[:].rearrange("p (g m) -> p g m", g=1), axis=AX.X)

                # ---- regression smooth-L1 (negated sums) ----
                dd = it("dd"); nsl = it("nsl"); za = it("za")

                def huber_neg(first, d):
                    nc.scalar.activation(tmp[:], d, ACT.Abs)
                    nc.vector.tensor_scalar(tmp2[:], tmp[:], 1.0, None, ALU.min)  # z
                    nc.vector.tensor_mul(za[:], tmp2[:], tmp[:])                  # z*a
                    nc.vector.tensor_mul(tmp2[:], tmp2[:], tmp2[:])               # z^2
                    nc.vector.scalar_tensor_tensor(
                        tmp[:], tmp2[:], 0.5, za[:], ALU.mult, ALU.subtract)      # .5z^2-za
                    if first:
                        nc.vector.tensor_copy(nsl[:], tmp[:])
                    else:
                        nc.vector.tensor_add(nsl[:], nsl[:], tmp[:])

                for comp, (q, acp, invp) in enumerate(
                        ((0, A_CX, A_I2W), (1, A_CY, A_I2H))):
                    nc.vector.tensor_tensor(_v3(tmp[:]), g5(q), _v3(ancp(acp)),
                                            ALU.subtract)
                    nc.vector.tensor_tensor(tmp[:], tmp[:], ancp(invp), ALU.mult)
                    nc.vector.tensor_scalar(tmp2[:], regp(comp), 0.5, None, ALU.subtract)
                    nc.vector.tensor_sub(dd[:], tmp2[:], tmp[:])
                    huber_neg(comp == 0, dd[:])
                for comp, (q, lgp) in enumerate(((2, A_LW), (3, A_LH))):
                    nc.vector.tensor_tensor(_v3(tmp[:]), g5(q), _v3(ancp(lgp)),
                                            ALU.subtract)
                    nc.vector.tensor_sub(dd[:], regp(comp + 2), tmp[:])
                    huber_neg(False, dd[:])
                nc.vector.tensor_mul(scr[:], nsl[:], posf[:])
                nc.vector.reduce_sum(ot[:, 5:6],
                                     scr[:].rearrange("p (g m) -> p g m", g=1), axis=AX.X)

                nc.sync.dma_start(res_d[b], ot[:])
    nc.compile()
    return nc


_NC_CACHE = None


def _get_nc():
    global _NC_CACHE
    if _NC_CACHE is None:
        _NC_CACHE = build_program()
    return _NC_CACHE


def prep_inputs(cls_output, reg_output, anchors, gt_boxes, gt_labels, num_boxes):
    """Host-side shard + derived-plane prep. Returns (in_maps, num_boxes)."""
    B = cls_output.shape[0]
    cls_output = np.asarray(cls_output, np.float32)
    reg_output = np.asarray(reg_output, np.float32)
    anchors = np.asarray(anchors, np.float32)
    gt_boxes = np.asarray(gt_boxes, np.float32)
    gt_labels = np.asarray(gt_labels)
    num_boxes = np.asarray(num_boxes)

    aw = anchors[:, 2] - anchors[:, 0]
    ah = anchors[:, 3] - anchors[:, 1]
    acx = anchors[:, 0] + 0.5 * aw
    acy = anchors[:, 1] + 0.5 * ah
    planes = np.stack([
        acx - aw / 4.0, acy - ah / 4.0, aw / 2.0, ah / 2.0, aw, ah,
        2.0 / aw, 2.0 / ah, np.log(aw), np.log(ah), acx, acy,
    ], axis=0).astype(np.float32)                       # [12, N]
    anc_host = planes.reshape(NANC, P, G).transpose(1, 0, 2).reshape(P, NANC * G)

    gx1 = gt_boxes[..., 0]; gy1 = gt_boxes[..., 1]
    gx2 = gt_boxes[..., 2]; gy2 = gt_boxes[..., 3]
    area2 = (gx2 - gx1) * (gy2 - gy1)
    valid = np.arange(M)[None, :] < num_boxes[:, None]
    area2 = np.where(valid, area2, np.float32(1e30)).astype(np.float32)
    gt_all = np.stack([gx1, gy1, gx2, gy2, area2], axis=1)        # [B,5,M]
    gt_host = np.broadcast_to(gt_all[:, None, :, :], (B, P, NGT, M)) \
        .reshape(B, P, NGT * M).astype(np.float32)

    gw = gx2 - gx1; gh = gy2 - gy1
    gcx = gx1 + np.float32(0.5) * gw
    gcy = gy1 + np.float32(0.5) * gh
    lgw = np.log(np.maximum(gw, np.float32(1e-6)))
    lgh = np.log(np.maximum(gh, np.float32(1e-6)))
    gt5_host = np.zeros((B, 64, 8), np.float32)
    gt5_host[:, :M, 0] = gcx; gt5_host[:, :M, 1] = gcy
    gt5_host[:, :M, 2] = lgw; gt5_host[:, :M, 3] = lgh
    gt5_host[:, :M, 4] = gt_labels.astype(np.float32)

    cls_host = cls_output.reshape(B, C, P, G).transpose(0, 2, 1, 3) \
        .reshape(B, P, C * G).copy()
    reg_host = reg_output.reshape(B, 4, P, G).transpose(0, 2, 1, 3) \
        .reshape(B, P, 4 * G).copy()
    iden = np.eye(P, dtype=np.float32)

    in_maps = []
    for core in range(NCORES):
        s = slice(core * BPC, (core + 1) * BPC)
        in_maps.append({
            "cls": np.ascontiguousarray(cls_host[s]),
            "reg": np.ascontiguousarray(reg_host[s]),
            "anc": anc_host,
            "gt": np.ascontiguousarray(gt_host[s]),
            "gt5": np.ascontiguousarray(gt5_host[s]),
            "iden": iden,
        })
    return in_maps, num_boxes


def finish(res_all, num_boxes):
    """res_all: [B, P, NOUT] partial sums. Reproduce reference scalar combine."""
    s = res_all.sum(axis=1).astype(np.float32)          # [B, NOUT]
    npos, nneg, ce_bg_sum, ce_tgt_pos, ce_bg_neg, neg_sl = (s[:, i] for i in range(6))
    sl_pos = -neg_sl
    has = num_boxes > 0
    cls_pos = np.where(npos > 0, ce_tgt_pos / np.maximum(npos, 1.0), 0.0)
    cls_neg = np.where(nneg > 0, ce_bg_neg / np.maximum(nneg, 1.0), 0.0)
    cls_losses = np.where(has, cls_pos + cls_neg, ce_bg_sum / np.float32(N))
    reg_losses = np.where(npos > 0, sl_pos / np.maximum(npos * 4.0, 1.0), 0.0)
    total_pos = npos.sum(dtype=np.float32)
    cls_final = np.float32(cls_losses.astype(np.float32).mean())
    reg_final = np.float32(reg_losses.astype(np.float32).sum() / max(total_pos, 1.0))
    total = np.float32(cls_final + reg_final)
    return total, cls_final, reg_final, np.float32(total_pos)


def kernel(cls_output, reg_output, anchors, gt_boxes, gt_labels, num_boxes):
    nc = _get_nc()
    in_maps, num_boxes = prep_inputs(
        cls_output, reg_output, anchors, gt_boxes, gt_labels, num_boxes)
    out = run_bass_kernel_spmd(nc, in_maps, list(range(NCORES)))
    res_all = np.concatenate([np.asarray(r["res"]) for r in out.results], axis=0)
    return finish(res_all, num_boxes)


# revision 11
# speedup vs baseline: 1.7980x; 1.7980x over previous
"""DetectionLoss Trainium2 kernel: 8-core data-parallel (4 images/core).

Device computes, per image, partial sums over anchors ([128,6] per image):
  [npos, nneg, sum(ce_bg), sum(ce_tgt*posf), sum(ce_bg*negf), -sum(sl*posf)]
Host finishes the scalar combine exactly as the reference does.

Matched-GT gather runs on the PE: transpose the argmax tie-mask, then
matmul against per-GT [gcx, gcy, log(gw), log(gh), label] columns.
"""
import os
import sys
import numpy as np

sys.path.insert(0, "/opt/trn_rl_repo")

import concourse.bass as bass
import concourse.bacc as bacc
import concourse.mybir as mybir
from concourse import tile
from concourse.bass_utils import run_bass_kernel_spmd

F32 = mybir.dt.float32
ALU = mybir.AluOpType
ACT = mybir.ActivationFunctionType
AX = mybir.AxisListType

P = 128          # partitions
G = 200          # free columns per anchor plane (N = P*G = 25600)
N = P * G
M = 50           # max GT boxes
C = 8            # classes
BPC = 4          # images per core
NCORES = 8
GPC = 25         # groups per pair-stage chunk
NCHUNK = G // GPC
NQ = 5           # gathered per-GT quantities

# anchor plane indices in the "anc" DRAM tensor
A_CXM, A_CYM, A_WH, A_HH, A_W, A_H, A_I2W, A_I2H, A_LW, A_LH, A_CX, A_CY = range(12)
NANC = 12
# gt plane indices (each M wide) in "gt"
G_X1, G_Y1, G_X2, G_Y2, G_A2 = range(5)
NGT = 5
NOUT = 6


def _rep_last(ap, n):
    """[..., d] -> [..., d, n] with step-0 broadcast."""
    return bass.AP(ap.tensor, ap.offset, list(ap.ap) + [[0, n]])


def _rep_mid(ap, n):
    """[p, d] -> [p, n, d] with step-0 broadcast."""
    a = list(ap.ap)
    return bass.AP(ap.tensor, ap.offset, [a[0], [0, n]] + a[1:])


def _v3(ap2d):
    """[128, 200] plane -> [128, 8, 25]."""
    return ap2d.rearrange("p (u v) -> p u v", v=GPC)


def build_program():
    nc = bacc.Bacc(None, target_bir_lowering=False)
    cls_d = nc.dram_tensor("cls", [BPC, P, C * G], F32, kind="ExternalInput")
    reg_d = nc.dram_tensor("reg", [BPC, P, 4 * G], F32, kind="ExternalInput")
    anc_d = nc.dram_tensor("anc", [P, NANC * G], F32, kind="ExternalInput")
    gt_d = nc.dram_tensor("gt", [BPC, P, NGT * M], F32, kind="ExternalInput")
    gt5_d = nc.dram_tensor("gt5", [BPC, 64, 8], F32, kind="ExternalInput")
    iden_d = nc.dram_tensor("iden", [P, P], F32, kind="ExternalInput")
    res_d = nc.dram_tensor("res", [BPC, P, NOUT], F32, kind="ExternalOutput")

    with tile.TileContext(nc) as tc:
        with (
            tc.tile_pool(name="const", bufs=1) as cpool,
            tc.tile_pool(name="img", bufs=2) as ipool,
            tc.tile_pool(name="work", bufs=2) as wpool,
            tc.tile_pool(name="psum", bufs=2, space="PSUM") as ppool,
            tc.tile_pool(name="out", bufs=2) as opool,
        ):
            anc = cpool.tile([P, NANC * G], F32)
            nc.sync.dma_start(anc[:], anc_d[:])
            iden = cpool.tile([P, P], F32)
            nc.sync.dma_start(iden[:], iden_d[:])

            def ancp(k):
                return anc[:, k * G:(k + 1) * G]

            for b in [bb for _ in range(int(os.environ.get('DETLOSS_REPS', '1'))) for bb in range(BPC)]:
                ct = ipool.tile([P, C * G], F32, tag="ct", name="ct")
                nc.sync.dma_start(ct[:], cls_d[b])
                rt = ipool.tile([P, 4 * G], F32, tag="rt", name="rt")
                nc.sync.dma_start(rt[:], reg_d[b])
                gtt = ipool.tile([P, NGT * M], F32, tag="gtt", name="gtt")
                nc.sync.dma_start(gtt[:], gt_d[b])
                gtq = ipool.tile([P, 8], F32, tag="gtq", name="gtq")
                nc.sync.dma_start(gtq[0:64, :], gt5_d[b])
                # gathered per-GT quantities, chunk-major [8 x [128, 25*5]]
                pg5 = ipool.tile([P, NCHUNK * GPC * NQ], F32, tag="pg5", name="pg5")

                def g5(q):
                    """[128, 8, 25] strided view of gathered quantity q."""
                    a = pg5[:, :]
                    return bass.AP(a.tensor, a.offset + q,
                                   [a.ap[0], [GPC * NQ, NCHUNK], [NQ, GPC]])

                def clsp(k):
                    return ct[:, k * G:(k + 1) * G]

                def regp(k):
                    return rt[:, k * G:(k + 1) * G]

                def gtp(k):
                    return gtt[:, k * M:(k + 1) * M]

                def it(tag):
                    return ipool.tile([P, G], F32, tag=tag, name=tag)

                # ---- decode boxes ----
                cx = it("cx"); cy = it("cy"); w = it("w"); h = it("h")
                ew = it("ew"); hw = it("hw")
                x1 = it("x1"); x2 = it("x2"); y1 = it("y1"); y2 = it("y2")
                a1 = it("a1")
                nc.vector.tensor_tensor(cx[:], regp(0), ancp(A_WH), ALU.mult)
                nc.vector.tensor_tensor(cx[:], cx[:], ancp(A_CXM), ALU.add)
                nc.vector.tensor_tensor(cy[:], regp(1), ancp(A_HH), ALU.mult)
                nc.vector.tensor_tensor(cy[:], cy[:], ancp(A_CYM), ALU.add)
                nc.scalar.activation(ew[:], regp(2), ACT.Exp)
                nc.vector.tensor_tensor(w[:], ew[:], ancp(A_W), ALU.mult)
                nc.scalar.activation(ew[:], regp(3), ACT.Exp)
                nc.vector.tensor_tensor(h[:], ew[:], ancp(A_H), ALU.mult)
                nc.scalar.activation(hw[:], w[:], ACT.Copy, scale=0.5)
                nc.vector.tensor_sub(x1[:], cx[:], hw[:])
                nc.vector.tensor_add(x2[:], cx[:], hw[:])
                nc.scalar.activation(hw[:], h[:], ACT.Copy, scale=0.5)
                nc.vector.tensor_sub(y1[:], cy[:], hw[:])
                nc.vector.tensor_add(y2[:], cy[:], hw[:])
                nc.vector.tensor_mul(a1[:], w[:], h[:])

                # ---- pair stage: per-anchor max IoU + matched-GT gather ----
                mx = it("mx")
                gx1b = _rep_mid(gtp(G_X1), GPC)
                gy1b = _rep_mid(gtp(G_Y1), GPC)
                gx2b = _rep_mid(gtp(G_X2), GPC)
                gy2b = _rep_mid(gtp(G_Y2), GPC)
                a2b = _rep_mid(gtp(G_A2), GPC)

                for k in range(NCHUNK):
                    g0 = k * GPC
                    sl = slice(g0, g0 + GPC)

                    def wt(tag):
                        t = wpool.tile([P, GPC * M], F32, tag=tag, name=tag)
                        return t, t[:].rearrange("p (g m) -> p g m", m=M)

                    ta, tav = wt("ta"); tb, tbv = wt("tb"); tcn, tcv = wt("tc")
                    td, tdv = wt("td"); te, tev = wt("te"); tf, tfv = wt("tf")

                    nc.vector.tensor_tensor(tav, _rep_last(x1[:, sl], M), gx1b, ALU.max)
                    nc.vector.tensor_tensor(tbv, _rep_last(x2[:, sl], M), gx2b, ALU.min)
                    nc.gpsimd.tensor_tensor(tcv, tbv, tav, ALU.subtract)
                    nc.vector.tensor_tensor(tav, _rep_last(y1[:, sl], M), gy1b, ALU.max)
                    nc.vector.tensor_tensor(tbv, _rep_last(y2[:, sl], M), gy2b, ALU.min)
                    nc.gpsimd.tensor_tensor(tdv, tbv, tav, ALU.subtract)
                    nc.scalar.activation(ta[:], tcn[:], ACT.Relu)
                    nc.scalar.activation(tb[:], td[:], ACT.Relu)
                    nc.vector.tensor_mul(tcn[:], ta[:], tb[:])          # inter
                    nc.gpsimd.tensor_tensor(tdv, _rep_last(a1[:, sl], M), a2b, ALU.add)
                    nc.vector.tensor_sub(te[:], td[:], tcn[:])          # union
                    nc.vector.reciprocal(tb[:], te[:])
                    nc.vector.tensor_mul(tf[:], tcn[:], tb[:])          # iou
                    nc.vector.reduce_max(mx[:, sl], tfv, axis=AX.X)
                    nc.vector.tensor_tensor(tav, tfv, _rep_last(mx[:, sl], M),
                                            ALU.is_equal)               # tie-mask
                    # PE gather: out[anchor, q] = sum_gt mask * gtq
                    pout = ppool.tile([P, GPC * NQ], F32, tag="pout", name="pout")
                    for g in range(GPC):
                        w0 = g * M
                        psT = ppool.tile([M, P], F32, tag="psT", name="psT")
                        nc.tensor.transpose(psT[:], ta[:, w0:w0 + M], iden[:])
                        ohT = wpool.tile([M, P], F32, tag="ohT", name="ohT")
                        nc.scalar.activation(ohT[:], psT[:], ACT.Copy)
                        nc.tensor.matmul(pout[:, g * NQ:(g + 1) * NQ],
                                         ohT[:], gtq[0:M, 0:NQ])
                    nc.scalar.activation(pg5[:, k * GPC * NQ:(k + 1) * GPC * NQ],
                                         pout[:], ACT.Copy)

                # ---- classification ----
                acc = it("acc"); tmp = it("tmp"); tmp2 = it("tmp2")
                lse = it("lse"); bgt = it("bgt"); xt = it("xt"); tgtt = it("tgtt")
                posf = it("posf"); negf = it("negf"); scr = it("scr")
                for c in range(C):
                    if c == 0:
                        nc.scalar.activation(acc[:], clsp(0), ACT.Exp)
                    else:
                        nc.scalar.activation(tmp2[:], clsp(c), ACT.Exp)
                        nc.vector.tensor_add(acc[:], acc[:], tmp2[:])
                nc.scalar.activation(lse[:], acc[:], ACT.Ln)
                nc.vector.tensor_sub(bgt[:], lse[:], clsp(0))           # ce_bg
                labv = g5(4)
                for c in range(C):
                    if c == 0:
                        nc.vector.scalar_tensor_tensor(
                            _v3(xt[:]), labv, 0.0, _v3(clsp(0)), ALU.is_equal, ALU.mult)
                    else:
                        nc.vector.scalar_tensor_tensor(
                            _v3(tmp[:]), labv, float(c), _v3(clsp(c)),
                            ALU.is_equal, ALU.mult)
                        nc.vector.tensor_add(xt[:], xt[:], tmp[:])
                nc.vector.tensor_sub(tgtt[:], lse[:], xt[:])            # ce_tgt
                nc.vector.tensor_scalar(posf[:], mx[:], 0.25, None, ALU.is_ge)
                nc.vector.tensor_scalar(negf[:], mx[:], 0.1, None, ALU.is_lt)

                ot = opool.tile([P, NOUT], F32, tag="ot", name="ot")
                nc.scalar.activation(scr[:], posf[:], ACT.Copy, accum_out=ot[:, 0:1])
                nc.scalar.activation(scr[:], negf[:], ACT.Copy, accum_out=ot[:, 1:2])
                nc.scalar.activation(scr[:], bgt[:], ACT.Copy, accum_out=ot[:, 2:3])
                nc.vector.tensor_mul(scr[:], tgtt[:], posf[:])
                nc.vector.reduce_sum(ot[:, 3:4],
                                     scr[:].rearrange("p (g m) -> p g m", g=1), axis=AX.X)
                nc.vector.tensor_mul(scr[:], bgt[:], negf[:])
                nc.vector.reduce_sum(ot[:, 4:5],
                                     scr[:].rearrange("p (g m) -> p g m", g=1), axis=AX.X)

                # ---- regression smooth-L1 (negated sums) ----
                dd = it("dd"); nsl = it("nsl"); za = it("za")

                def huber_neg(first, d):
                    nc.scalar.activation(tmp[:], d, ACT.Abs)
                    nc.vector.tensor_scalar(tmp2[:], tmp[:], 1.0, None, ALU.min)  # z
                    nc.vector.tensor_mul(za[:], tmp2[:], tmp[:])                  # z*a
                    nc.vector.tensor_mul(tmp2[:], tmp2[:], tmp2[:])               # z^2
                    nc.vector.scalar_tensor_tensor(
                        tmp[:], tmp2[:], 0.5, za[:], ALU.mult, ALU.subtract)      # .5z^2-za
                    if first:
                        nc.vector.tensor_copy(nsl[:], tmp[:])
                    else:
                        nc.vector.tensor_add(nsl[:], nsl[:], tmp[:])

                for comp, (q, acp, invp) in enumerate(
                        ((0, A_CX, A_I2W), (1, A_CY, A_I2H))):
                    nc.vector.tensor_tensor(_v3(tmp[:]), g5(q), _v3(ancp(acp)),
                                            ALU.subtract)
                    nc.vector.tensor_tensor(tmp[:], tmp[:], ancp(invp), ALU.mult)
                    nc.vector.tensor_scalar(tmp2[:], regp(comp), 0.5, None, ALU.subtract)
                    nc.vector.tensor_sub(dd[:], tmp2[:], tmp[:])
                    huber_neg(comp == 0, dd[:])
                for comp, (q, lgp) in enumerate(((2, A_LW), (3, A_LH))):
                    nc.vector.tensor_tensor(_v3(tmp[:]), g5(q), _v3(ancp(lgp)),
                                            ALU.subtract)
                    nc.vector.tensor_sub(dd[:], regp(comp + 2), tmp[:])
                    huber_neg(False, dd[:])
                nc.vector.tensor_mul(scr[:], nsl[:], posf[:])
                nc.vector.reduce_sum(ot[:, 5:6],
                                     scr[:].rearrange("p (g m) -> p g m", g=1), axis=AX.X)

                nc.sync.dma_start(res_d[b], ot[:])
    nc.compile()
    return nc


_NC_CACHE = None


def _get_nc():
    global _NC_CACHE
    if _NC_CACHE is None:
        _NC_CACHE = build_program()
    return _NC_CACHE


def prep_inputs(cls_output, reg_output, anchors, gt_boxes, gt_labels, num_boxes):
    """Host-side shard + derived-plane prep. Returns (in_maps, num_boxes)."""
    B = cls_output.shape[0]
    cls_output = np.asarray(cls_output, np.float32)
    reg_output = np.asarray(reg_output, np.float32)
    anchors = np.asarray(anchors, np.float32)
    gt_boxes = np.asarray(gt_boxes, np.float32)
    gt_labels = np.asarray(gt_labels)
    num_boxes = np.asarray(num_boxes)

    aw = anchors[:, 2] - anchors[:, 0]
    ah = anchors[:, 3] - anchors[:, 1]
    acx = anchors[:, 0] + 0.5 * aw
    acy = anchors[:, 1] + 0.5 * ah
    planes = np.stack([
        acx - aw / 4.0, acy - ah / 4.0, aw / 2.0, ah / 2.0, aw, ah,
        2.0 / aw, 2.0 / ah, np.log(aw), np.log(ah), acx, acy,
    ], axis=0).astype(np.float32)                       # [12, N]
    anc_host = planes.reshape(NANC, P, G).transpose(1, 0, 2).reshape(P, NANC * G)

    gx1 = gt_boxes[..., 0]; gy1 = gt_boxes[..., 1]
    gx2 = gt_boxes[..., 2]; gy2 = gt_boxes[..., 3]
    area2 = (gx2 - gx1) * (gy2 - gy1)
    valid = np.arange(M)[None, :] < num_boxes[:, None]
    area2 = np.where(valid, area2, np.float32(1e30)).astype(np.float32)
    gt_all = np.stack([gx1, gy1, gx2, gy2, area2], axis=1)        # [B,5,M]
    gt_host = np.broadcast_to(gt_all[:, None, :, :], (B, P, NGT, M)) \
        .reshape(B, P, NGT * M).astype(np.float32)

    gw = gx2 - gx1; gh = gy2 - gy1
    gcx = gx1 + np.float32(0.5) * gw
    gcy = gy1 + np.float32(0.5) * gh
    lgw = np.log(np.maximum(gw, np.float32(1e-6)))
    lgh = np.log(np.maximum(gh, np.float32(1e-6)))
    gt5_host = np.zeros((B, 64, 8), np.float32)
    gt5_host[:, :M, 0] = gcx; gt5_host[:, :M, 1] = gcy
    gt5_host[:, :M, 2] = lgw; gt5_host[:, :M, 3] = lgh
    gt5_host[:, :M, 4] = gt_labels.astype(np.float32)

    cls_host = cls_output.reshape(B, C, P, G).transpose(0, 2, 1, 3) \
        .reshape(B, P, C * G).copy()
    reg_host = reg_output.reshape(B, 4, P, G).transpose(0, 2, 1, 3) \
        .reshape(B, P, 4 * G).copy()
    iden = np.eye(P, dtype=np.float32)

    in_maps = []
    for core in range(NCORES):
        s = slice(core * BPC, (core + 1) * BPC)
        in_maps.append({
            "cls": np.ascontiguousarray(cls_host[s]),
            "reg": np.ascontiguousarray(reg_host[s]),
            "anc": anc_host,
            "gt": np.ascontiguousarray(gt_host[s]),
            "gt5": np.ascontiguousarray(gt5_host[s]),
            "iden": iden,
        })
    return in_maps, num_boxes


def finish(res_all, num_boxes):
    """res_all: [B, P, NOUT] partial sums. Reproduce reference scalar combine."""
    s = res_all.sum(axis=1).astype(np.float32)          # [B, NOUT]
    npos, nneg, ce_bg_sum, ce_tgt_pos, ce_bg_neg, neg_sl = (s[:, i] for i in range(6))
    sl_pos = -neg_sl
    has = num_boxes > 0
    cls_pos = np.where(npos > 0, ce_tgt_pos / np.maximum(npos, 1.0), 0.0)
    cls_neg = np.where(nneg > 0, ce_bg_neg / np.maximum(nneg, 1.0), 0.0)
    cls_losses = np.where(has, cls_pos + cls_neg, ce_bg_sum / np.float32(N))
    reg_losses = np.where(npos > 0, sl_pos / np.maximum(npos * 4.0, 1.0), 0.0)
    total_pos = npos.sum(dtype=np.float32)
    cls_final = np.float32(cls_losses.astype(np.float32).mean())
    reg_final = np.float32(reg_losses.astype(np.float32).sum() / max(total_pos, 1.0))
    total = np.float32(cls_final + reg_final)
    return total, cls_final, reg_final, np.float32(total_pos)


def kernel(cls_output, reg_output, anchors, gt_boxes, gt_labels, num_boxes):
    nc = _get_nc()
    in_maps, num_boxes = prep_inputs(
        cls_output, reg_output, anchors, gt_boxes, gt_labels, num_boxes)
    out = run_bass_kernel_spmd(nc, in_maps, list(range(NCORES)))
    res_all = np.concatenate([np.asarray(r["res"]) for r in out.results], axis=0)
    return finish(res_all, num_boxes)
